# revision 7
# baseline (speedup 1.0000x reference)
"""Trainium2 Bass kernel for gated-attention pooling (B=8, N=8192, L=1024, D=256).

Reference computation (per batch b):
    a = tanh(x Wa + ba); g = sigmoid(x Wb + bb)
    A = (a*g) Wc + bc                      # [N] scores
    w = softmax(mask(A, lengths))          # over N
    out[b] = w @ x[b]                      # [L]

Strategy (v2):
  - softmax decomposes into unnormalized sums  P = sum_n exp(A_n) x_n  and
    S = sum_n exp(A_n); work shards across 8 cores at 512-row super-tile (ST)
    granularity.  The host packs valid rows per batch into STs and deals them
    to the cores; padding rows are zero, so they contribute nothing to P, and
    the host sums only the valid prefix of the exp row for S (no device mask).
  - x is uploaded pre-transposed (L-major) TWICE: an fp8(e4m3) copy feeding
    DoubleRow score matmuls (2x PE rate) and a bf16 copy feeding the pooling
    (DVE 2x_1P rate needs 2-byte operands).  A few STs per core (NB of S) use
    bf16 scores instead (no fp8 upload for them) to balance PE time against
    the ~358 GB/s/core HBM bandwidth.
  - sigmoid(z) = 0.5*tanh(z/2) + 0.5 keeps ACT inside one table set; the 0.5
    factors fold into Wc and the tanh input scale.  fp8 weights carry a x64
    pre-scale, undone by the tanh input scale.
  - Per ST: scores -> tanh -> m = (t+1)*a (DVE) -> A-row matmul with a
    column-replicated Wc stationary, so the PSUM result [128, ST] already
    holds the score row broadcast across all 128 partitions -> ACT exp gives
    u broadcast [128, ST] in one op (no separate broadcast matmul, no copy,
    no mask) -> DVE scalar_tensor_tensor with accum_out pools each L-chunk.
  - Outputs per core: pp [128, S*LC] (per-ST per-L-chunk partial pools) and
    urow [1, S*ST] (bf16 unnormalized weights, DMA'd per ST from u's first
    partition); the host reduces per batch and divides.

MODE:
  "v2" (default) - hybrid fp8-DoubleRow/bf16 scores + bf16 pooling.
  "f32r"/"bf16"/"fp8" - previous-generation single/dual upload variants.
"""

import numpy as np
import ml_dtypes

BF16 = ml_dtypes.bfloat16
FP8 = ml_dtypes.float8_e4m3
B, N, L, D = 8, 8192, 1024, 256
ST = 512          # rows per super-tile
LC = L // 128     # 8 L-chunks

MODE = "v2"       # "v3" | "v2" | "f32r" | "bf16" | "fp8"
K_GP = 1          # v3: pooling chunks per group routed gpsimd-TT + ACT-accum
ABLATE = ""       # v3 timing ablations: "no_pool" | "no_scores" | "no_dma" | ""
NB = 3            # v2: number of bf16-score STs per core (rest use fp8 scores)
POOL_CHUNKS = 8   # v2: L-chunks pooled per ST (ablation knob; 8 = correct)
PAIRED = False    # v2: process STs in pairs with shared score stationaries
LDW_OPT = False   # rewrite walrus --enable-ldw-opt=false -> true
POOL_NACT = 0     # v2: pooling chunks routed as DVE-product + ACT-reduce


def _patch_ldw_opt():
    """Rewrite the hardcoded --enable-ldw-opt=false walrus flag so identical
    consecutive LDWEIGHTS can be deduped (pairs with the PAIRED schedule)."""
    import concourse.bass_utils as bu

    if getattr(bu.run_command, "_ldw_patched", False):
        return
    orig = bu.run_command

    def patched(argv, **kwargs):
        argv = [
            a.replace("--enable-ldw-opt=false", "--enable-ldw-opt=true")
            if isinstance(a, str) else a
            for a in argv
        ]
        return orig(argv, **kwargs)

    patched._ldw_patched = True
    bu.run_command = patched
W_SCALE = 64.0    # fp8 weight pre-scale, undone by the tanh input scale
XBUFS = 4         # x-tile buffering (deep prefetch)
WBUFS = 5         # working-tile buffering (act/m/u/scratch)

_cache = {}


def _split_multiwait(nc, max_waits=1):
    """This container's walrus rejects instructions carrying more than a
    couple of semaphore waits ("Too many sync wait commands").  Split extras
    into same-engine single-wait NoOps placed immediately before."""
    import concourse.mybir as mybir

    for f in nc.m.functions:
        for bb in f.blocks:
            insts = bb.instructions  # live list
            new = []
            for inst in insts:
                si = inst.sync_info
                if si is not None and len(si.on_wait) > max_waits:
                    waits = list(si.on_wait)
                    for k, w in enumerate(waits[:-max_waits]):
                        nop = mybir.InstNoOp(
                            name=f"{inst.name}-wsplit{k}", ins=[], outs=[]
                        )
                        nop.engine = inst.engine
                        nop.debug = inst.debug
                        nop.sync_info = mybir.SyncInfo(on_wait=[w], on_update=[])
                        new.append(nop)
                    inst.sync_info = mybir.SyncInfo(
                        on_wait=waits[-max_waits:], on_update=list(si.on_update)
                    )
                new.append(inst)
            insts[:] = new


def _bf16_sts(S, nb):
    """Which STs use bf16 scores: spread evenly through the schedule."""
    if nb <= 0:
        return set()
    return {int((i + 0.5) * S / nb) for i in range(nb)}


def _build_v2(S, reps=1, nb=None, split_multiwait=True):
    if nb is None:
        nb = NB
    import concourse.bass as bass
    import concourse.mybir as mybir
    import concourse.tile as tile
    from contextlib import ExitStack

    f32 = mybir.dt.float32
    bf16 = mybir.dt.bfloat16
    fp8e4 = mybir.dt.float8e4
    ACT = mybir.ActivationFunctionType
    ALU = mybir.AluOpType

    bfset = _bf16_sts(S, nb)

    nc = bass.Bass()
    xt8_d = nc.declare_dram_parameter("xt8", [S, 128, LC, ST], fp8e4, isOutput=False)
    xtb_d = nc.declare_dram_parameter("xtb", [S, 128, LC, ST], bf16, isOutput=False)
    w8_d = nc.declare_dram_parameter("w8", [128, 32 * 128], fp8e4, isOutput=False)
    wb_d = nc.declare_dram_parameter("wb", [128, 32 * 128], bf16, isOutput=False)
    bcol_d = nc.declare_dram_parameter("bcol", [128, 4], f32, isOutput=False)
    wrep_d = nc.declare_dram_parameter("wrep", [128, 2 * 128], bf16, isOutput=False)
    bcs_d = nc.declare_dram_parameter("bcs", [128, 1], f32, isOutput=False)
    pp_d = nc.declare_dram_parameter("pp", [128, S * LC], f32, isOutput=True)
    urow_d = nc.declare_dram_parameter("urow", [1, S * ST], bf16, isOutput=True)

    with tile.TileContext(nc) as tc, ExitStack() as ctx:
        const = ctx.enter_context(tc.tile_pool(name="const", bufs=1))
        outp = ctx.enter_context(tc.tile_pool(name="outp", bufs=1))
        xp8 = ctx.enter_context(tc.tile_pool(name="x8", bufs=XBUFS))
        xpb = ctx.enter_context(tc.tile_pool(name="xb", bufs=XBUFS))
        apool = ctx.enter_context(tc.tile_pool(name="act", bufs=WBUFS))
        mpool = ctx.enter_context(tc.tile_pool(name="m", bufs=WBUFS))
        upool = ctx.enter_context(tc.tile_pool(name="u", bufs=WBUFS))
        spool = ctx.enter_context(tc.tile_pool(name="scr", bufs=WBUFS))
        scp = ctx.enter_context(
            tc.tile_pool(name="scp", bufs=6 if PAIRED else 4, space="PSUM")
        )
        ap_ps = ctx.enter_context(tc.tile_pool(name="apps", bufs=2, space="PSUM"))

        w8_sb = const.tile([128, 32 * 128], fp8e4, tag="w8")
        nc.sync.dma_start(w8_sb[:], w8_d[:])
        wb_sb = const.tile([128, 32 * 128], bf16, tag="wb")
        nc.sync.dma_start(wb_sb[:], wb_d[:])
        bcol = const.tile([128, 4], f32, tag="bcol")
        nc.sync.dma_start(bcol[:], bcol_d[:])
        wrep = const.tile([128, 2 * 128], bf16, tag="wrep")
        nc.sync.dma_start(wrep[:], wrep_d[:])
        bcs = const.tile([128, 1], f32, tag="bcs")
        nc.sync.dma_start(bcs[:], bcs_d[:])

        pp_sb = outp.tile([128, S * LC], f32, tag="pp")

        w8_3d = w8_sb[:].rearrange("p (k m) -> p k m", k=32)

        def do_st(st):
            use8 = st not in bfset
            if use8:
                x8 = xp8.tile([128, LC * ST], fp8e4, tag="x8")
                nc.sync.dma_start(x8[:], xt8_d[st].rearrange("p c n -> p (c n)"))
                x8_3d = x8[:].rearrange("p (c n) -> p c n", c=LC)
            xb = xpb.tile([128, LC * ST], bf16, tag="xb")
            nc.sync.dma_start(xb[:], xtb_d[st].rearrange("p c n -> p (c n)"))

            # ---- scores: 4 weight groups (a0, a1, t0, t1) ----
            acts = []
            for wg in range(4):
                ps = scp.tile([128, ST], f32, tag="ps")
                if use8:
                    for l2 in range(LC // 2):
                        nc.tensor.matmul(
                            ps[:],
                            w8_3d[:, wg * LC + 2 * l2 : wg * LC + 2 * l2 + 2, :],
                            x8_3d[:, 2 * l2 : 2 * l2 + 2, :],
                            start=(l2 == 0),
                            stop=(l2 == (0 if ABLATE == "no_scores" else LC // 2 - 1)),
                            perf_mode=mybir.MatmulPerfMode.DoubleRow,
                        )
                else:
                    for lc in range(LC):
                        nc.tensor.matmul(
                            ps[:],
                            wb_sb[:, (wg * LC + lc) * 128 : (wg * LC + lc + 1) * 128],
                            xb[:, lc * ST : (lc + 1) * ST],
                            start=(lc == 0),
                            stop=(lc == LC - 1),
                        )
                dst = apool.tile([128, ST], bf16, tag=f"act{wg}")
                sc = (1.0 if wg < 2 else 0.5) / (W_SCALE if use8 else 1.0)
                nc.scalar.activation(
                    dst[:], ps[:], ACT.Tanh, bias=bcol[:, wg : wg + 1], scale=sc
                )
                acts.append(dst)

            a0, a1, t0, t1 = acts
            # ---- m = (t + 1) * a  (per D-chunk) ----
            m0 = mpool.tile([128, ST], bf16, tag="m0")
            nc.vector.scalar_tensor_tensor(m0[:], t0[:], 1.0, a0[:], ALU.add, ALU.mult)
            m1 = mpool.tile([128, ST], bf16, tag="m1")
            nc.vector.scalar_tensor_tensor(m1[:], t1[:], 1.0, a1[:], ALU.add, ALU.mult)
            # ---- A (broadcast to all partitions via column-replicated Wc) ----
            aps = ap_ps.tile([128, ST], f32, tag="A")
            nc.tensor.matmul(aps[:], wrep[:, 0:128], m0[:], start=True, stop=False)
            nc.tensor.matmul(aps[:], wrep[:, 128:256], m1[:], start=False, stop=True)
            # ---- u = exp(A + bc), already broadcast across partitions ----
            ubc = upool.tile([128, ST], bf16, tag="ubc")
            nc.scalar.activation(ubc[:], aps[:], ACT.Exp, bias=bcs[:, 0:1])
            nc.sync.dma_start(urow_d[0:1, st * ST : (st + 1) * ST], ubc[0:1, :])
            # ---- pooling: per L-chunk column sums into pp ----
            pool_st(st, xb, ubc)

        def tail_st(st, xb, acts):
            a0, a1, t0, t1 = acts
            m0 = mpool.tile([128, ST], bf16, tag="m0")
            nc.vector.scalar_tensor_tensor(m0[:], t0[:], 1.0, a0[:], ALU.add, ALU.mult)
            m1 = mpool.tile([128, ST], bf16, tag="m1")
            nc.vector.scalar_tensor_tensor(m1[:], t1[:], 1.0, a1[:], ALU.add, ALU.mult)
            aps = ap_ps.tile([128, ST], f32, tag="A")
            nc.tensor.matmul(aps[:], wrep[:, 0:128], m0[:], start=True, stop=False)
            nc.tensor.matmul(aps[:], wrep[:, 128:256], m1[:], start=False, stop=True)
            ubc = upool.tile([128, ST], bf16, tag="ubc")
            nc.scalar.activation(ubc[:], aps[:], ACT.Exp, bias=bcs[:, 0:1])
            nc.sync.dma_start(urow_d[0:1, st * ST : (st + 1) * ST], ubc[0:1, :])
            pool_st(st, xb, ubc)

        def pool_st(st, xb, ubc):
            for c in range(POOL_CHUNKS):
                col = pp_sb[:, st * LC + c : st * LC + c + 1]
                if c < K_GP:
                    # product on gpsimd, accumulate on ACT: frees DVE cycles
                    prod = spool.tile([128, ST], bf16, tag="prod")
                    nc.gpsimd.tensor_tensor(
                        prod[:], xb[:, c * ST : (c + 1) * ST], ubc[:], ALU.mult
                    )
                    dm = spool.tile([128, ST], bf16, tag="pdm")
                    nc.scalar.activation(dm[:], prod[:], ACT.Copy, accum_out=col)
                elif c < POOL_CHUNKS - POOL_NACT:
                    to = spool.tile([128, ST], bf16, tag="ttr")
                    nc.vector.scalar_tensor_tensor(
                        to[:],
                        xb[:, c * ST : (c + 1) * ST],
                        1.0,
                        ubc[:],
                        ALU.bypass,
                        ALU.mult,
                        accum_out=col,
                    )
                else:
                    to = spool.tile([128, ST], bf16, tag="ttr")
                    nc.vector.tensor_tensor(
                        to[:], xb[:, c * ST : (c + 1) * ST], ubc[:], ALU.mult
                    )
                    dm = spool.tile([128, ST], bf16, tag="dmy")
                    nc.scalar.activation(dm[:], to[:], ACT.Copy, accum_out=col)

        def do_pair(st0, st1):
            x8a = xp8.tile([128, LC * ST], fp8e4, tag="x8")
            nc.sync.dma_start(x8a[:], xt8_d[st0].rearrange("p c n -> p (c n)"))
            x8b = xp8.tile([128, LC * ST], fp8e4, tag="x8")
            nc.sync.dma_start(x8b[:], xt8_d[st1].rearrange("p c n -> p (c n)"))
            xba = xpb.tile([128, LC * ST], bf16, tag="xb")
            nc.sync.dma_start(xba[:], xtb_d[st0].rearrange("p c n -> p (c n)"))
            xbb = xpb.tile([128, LC * ST], bf16, tag="xb")
            nc.sync.dma_start(xbb[:], xtb_d[st1].rearrange("p c n -> p (c n)"))
            x3a = x8a[:].rearrange("p (c n) -> p c n", c=LC)
            x3b = x8b[:].rearrange("p (c n) -> p c n", c=LC)
            acts_a, acts_b = [], []
            for wg in range(4):
                psa = scp.tile([128, ST], f32, tag="ps")
                psb = scp.tile([128, ST], f32, tag="ps")
                for l2 in range(LC // 2):
                    w = w8_3d[:, wg * LC + 2 * l2 : wg * LC + 2 * l2 + 2, :]
                    nc.tensor.matmul(
                        psa[:], w, x3a[:, 2 * l2 : 2 * l2 + 2, :],
                        start=(l2 == 0), stop=(l2 == LC // 2 - 1),
                        perf_mode=mybir.MatmulPerfMode.DoubleRow,
                    )
                    nc.tensor.matmul(
                        psb[:], w, x3b[:, 2 * l2 : 2 * l2 + 2, :],
                        start=(l2 == 0), stop=(l2 == LC // 2 - 1),
                        perf_mode=mybir.MatmulPerfMode.DoubleRow,
                    )
                for ps, acts in ((psa, acts_a), (psb, acts_b)):
                    dst = apool.tile([128, ST], bf16, tag=f"act{wg}")
                    nc.scalar.activation(
                        dst[:], ps[:], ACT.Tanh,
                        bias=bcol[:, wg : wg + 1],
                        scale=(1.0 if wg < 2 else 0.5) / W_SCALE,
                    )
                    acts.append(dst)
            tail_st(st0, xba, acts_a)
            tail_st(st1, xbb, acts_b)

        if PAIRED:
            assert nb == 0 and S % 2 == 0, "PAIRED requires nb=0 and even S"
            for _rep in range(reps):
                for sp in range(S // 2):
                    do_pair(2 * sp, 2 * sp + 1)
        else:
            for _rep in range(reps):
                for st in range(S):
                    do_st(st)

        nc.sync.dma_start(pp_d[:], pp_sb[:])

    if split_multiwait:
        _split_multiwait(nc)
    return nc


def _prepare_v2(x, lengths, Wa, ba, Wb, bb, Wc, bc):
    """Host-side packing.  Returns (S, in_maps, slot_map) where slot_map[c][k]
    is (batch, row0, n_valid) or None for padding slots."""
    x = np.asarray(x, dtype=np.float32)
    lengths = np.asarray(lengths, dtype=np.int64)
    Wa = np.asarray(Wa, dtype=np.float32)
    ba = np.asarray(ba, dtype=np.float32)
    Wb = np.asarray(Wb, dtype=np.float32)
    bb = np.asarray(bb, dtype=np.float32)
    Wc = np.asarray(Wc, dtype=np.float32).reshape(D)
    bc = float(np.asarray(bc, dtype=np.float32).reshape(()))

    slots = []
    for b in range(B):
        ln = int(max(1, min(N, lengths[b])))
        for k in range((ln + ST - 1) // ST):
            slots.append((b, k * ST, min(ST, ln - k * ST)))
    n_cores = 8
    S = max(1, (len(slots) + n_cores - 1) // n_cores)

    # weights, shared by all cores: [K=128 (L-chunk), 32 chunks, M=128 (D)]
    wstack = np.zeros((128, 32, 128), dtype=np.float32)
    for g, W in enumerate((Wa, Wb)):
        for dc in range(2):
            for lc in range(LC):
                wstack[:, (g * 2 + dc) * LC + lc, :] = W[
                    lc * 128 : (lc + 1) * 128, dc * 128 : (dc + 1) * 128
                ]
    w8 = (wstack * W_SCALE).astype(FP8).reshape(128, 32 * 128)
    wb_ = wstack.astype(BF16).reshape(128, 32 * 128)
    bcol = np.stack(
        [ba[0:128], ba[128:256], bb[0:128] * 0.5, bb[128:256] * 0.5], axis=1
    ).astype(np.float32)
    # column-replicated Wc/2 stationaries: wrep[:, dc*128 + m] = Wc[dc*128 + p]/2
    wrep = np.zeros((128, 2, 128), dtype=np.float32)
    wrep[:, 0, :] = (Wc[0:128] * 0.5)[:, None]
    wrep[:, 1, :] = (Wc[128:256] * 0.5)[:, None]
    wrep = wrep.astype(BF16).reshape(128, 256)
    bcs = np.full((128, 1), bc, dtype=np.float32)

    in_maps = []
    slot_map = []
    xb_all = x.astype(BF16)
    for c in range(n_cores):
        xtb = np.zeros((S, 128, LC, ST), dtype=BF16)
        smap = []
        for k in range(S):
            g = c * S + k
            if g < len(slots):
                b, r0, nvalid = slots[g]
                slab = xb_all[b, r0 : r0 + nvalid, :]  # [nvalid, 1024]
                # xtb[k, p, lc, n] = x[b, r0+n, 128*lc+p]
                xtb[k, :, :, :nvalid] = slab.reshape(nvalid, LC, 128).transpose(2, 1, 0)
                smap.append((b, r0, nvalid))
            else:
                smap.append(None)
        im = {
            "xt8": xtb.astype(FP8),
            "xtb": xtb,
            "w8": w8,
            "wb": wb_,
            "bcol": bcol,
            "wrep": wrep,
            "bcs": bcs,
        }
        in_maps.append(im)
        slot_map.append(smap)
    return S, in_maps, slot_map


def _assemble_v2(results, S, slot_map):
    P = np.zeros((B, L), dtype=np.float64)
    Ssum = np.zeros(B, dtype=np.float64)
    for c in range(8):
        pp = np.asarray(results[c]["pp"], dtype=np.float64)  # [128, S*LC]
        ur = np.asarray(results[c]["urow"]).astype(np.float64)[0]  # [S*ST]
        for k, slot in enumerate(slot_map[c]):
            if slot is None:
                continue
            b, r0, nvalid = slot
            cols = pp[:, k * LC : (k + 1) * LC]  # [128, LC]
            P[b] += cols.T.reshape(L)
            Ssum[b] += ur[k * ST : k * ST + nvalid].sum()
    return (P / Ssum[:, None]).astype(np.float32)



# ---------------------------------------------------------------------------
# v3: paired 1024-wide groups; pooling offload to gpsimd+ACT; bf16 singles
# ---------------------------------------------------------------------------
def _build_v3(P, Q, reps=1):
    """P pair-groups (two same-batch 512-slots, width 1024, fp8 DR scores +
    bf16 pooling) and Q single-groups (width 512, bf16 scores, bf16-only
    upload).  Pooling: K_GP chunks/group via gpsimd product + ACT accum,
    rest DVE STT-accum."""
    import concourse.bass as bass
    import concourse.mybir as mybir
    import concourse.tile as tile
    from contextlib import ExitStack

    f32 = mybir.dt.float32
    bf16 = mybir.dt.bfloat16
    fp8e4 = mybir.dt.float8e4
    ACT = mybir.ActivationFunctionType
    ALU = mybir.AluOpType
    W = 1024  # pair width

    nc = bass.Bass()
    if P:
        xp8_d = nc.declare_dram_parameter("xp8", [P, 128, LC, W], fp8e4, isOutput=False)
        xpb_d = nc.declare_dram_parameter("xpb", [P, 128, LC, W], bf16, isOutput=False)
    if Q:
        xs_d = nc.declare_dram_parameter("xs", [Q, 128, LC, ST], bf16, isOutput=False)
    w8_d = nc.declare_dram_parameter("w8", [128, 32 * 128], fp8e4, isOutput=False)
    wb_d = nc.declare_dram_parameter("wb", [128, 32 * 128], bf16, isOutput=False)
    bcol_d = nc.declare_dram_parameter("bcol", [128, 4], f32, isOutput=False)
    wrep_d = nc.declare_dram_parameter("wrep", [128, 2 * 128], bf16, isOutput=False)
    bcs_d = nc.declare_dram_parameter("bcs", [128, 1], f32, isOutput=False)
    G = P + Q
    pp_d = nc.declare_dram_parameter("pp", [128, G * LC], f32, isOutput=True)
    urow_d = nc.declare_dram_parameter("urow", [1, P * W + Q * ST], bf16, isOutput=True)

    with tile.TileContext(nc) as tc, ExitStack() as ctx:
        const = ctx.enter_context(tc.tile_pool(name="const", bufs=1))
        outp = ctx.enter_context(tc.tile_pool(name="outp", bufs=1))
        xp8p = ctx.enter_context(tc.tile_pool(name="x8", bufs=3))
        xpbp = ctx.enter_context(tc.tile_pool(name="xb", bufs=3))
        apool = ctx.enter_context(tc.tile_pool(name="act", bufs=3))
        mpool = ctx.enter_context(tc.tile_pool(name="m", bufs=3))
        upool = ctx.enter_context(tc.tile_pool(name="u", bufs=3))
        spool = ctx.enter_context(tc.tile_pool(name="scr", bufs=4))
        gpool = ctx.enter_context(tc.tile_pool(name="gp", bufs=3))
        scp = ctx.enter_context(tc.tile_pool(name="scp", bufs=2, space="PSUM"))
        ap_ps = ctx.enter_context(tc.tile_pool(name="apps", bufs=2, space="PSUM"))

        w8_sb = const.tile([128, 32 * 128], fp8e4, tag="w8")
        nc.sync.dma_start(w8_sb[:], w8_d[:])
        wb_sb = const.tile([128, 32 * 128], bf16, tag="wb")
        nc.sync.dma_start(wb_sb[:], wb_d[:])
        bcol = const.tile([128, 4], f32, tag="bcol")
        nc.sync.dma_start(bcol[:], bcol_d[:])
        wrep = const.tile([128, 2 * 128], bf16, tag="wrep")
        nc.sync.dma_start(wrep[:], wrep_d[:])
        bcs = const.tile([128, 1], f32, tag="bcs")
        nc.sync.dma_start(bcs[:], bcs_d[:])

        pp_sb = outp.tile([128, G * LC], f32, tag="pp")
        if ABLATE == "no_pool":
            nc.vector.memset(pp_sb[:], 0.0)

        if ABLATE == "no_dma":
            x8c = const.tile([128, LC * 1024], fp8e4, tag="x8c")
            if P:
                nc.sync.dma_start(x8c[:], xp8_d[0].rearrange("p c n -> p (c n)"))
            else:
                nc.vector.memset(x8c[:], 0.0)
            xbc = const.tile([128, LC * 1024], bf16, tag="xbc")
            if P:
                nc.sync.dma_start(xbc[:], xpb_d[0].rearrange("p c n -> p (c n)"))
            else:
                nc.vector.memset(xbc[:], 0.0)

        w8_3d = w8_sb[:].rearrange("p (k m) -> p k m", k=32)

        def pool_tail(gidx, xsrc, ubc, width, uoff):
            # urow out
            nc.sync.dma_start(urow_d[0:1, uoff : uoff + width], ubc[0:1, :])
            if ABLATE == "no_pool":
                return
            for c in range(LC):
                col = pp_sb[:, gidx * LC + c : gidx * LC + c + 1]
                xs = xsrc[:, c * width : (c + 1) * width]
                if c < K_GP:
                    prod = gpool.tile([128, width], bf16, tag=f"prod{width}")
                    nc.gpsimd.tensor_tensor(prod[:], xs, ubc[:], ALU.mult)
                    dm = gpool.tile([128, width], bf16, tag=f"pdm{width}")
                    nc.scalar.activation(dm[:], prod[:], ACT.Copy, accum_out=col)
                else:
                    to = spool.tile([128, width], bf16, tag=f"ttr{width}")
                    nc.vector.scalar_tensor_tensor(
                        to[:], xs, 1.0, ubc[:], ALU.bypass, ALU.mult, accum_out=col
                    )

        def do_pair(g):
            if ABLATE == "no_dma":
                x8, xb = x8c, xbc
            else:
                x8 = xp8p.tile([128, LC * W], fp8e4, tag="x8")
                nc.sync.dma_start(x8[:], xp8_d[g].rearrange("p c n -> p (c n)"))
                xb = xpbp.tile([128, LC * W], bf16, tag="xb")
                nc.sync.dma_start(xb[:], xpb_d[g].rearrange("p c n -> p (c n)"))
            x8_3d = x8[:].rearrange("p (c n) -> p c n", c=LC)

            acts = []
            for wg in range(4):
                ps_t = scp.tile([128, W], f32, tag="ps_p")
                ps = ps_t[:]
                for l2 in range(1 if ABLATE == "no_scores" else LC // 2):
                    wsl = w8_3d[:, wg * LC + 2 * l2 : wg * LC + 2 * l2 + 2, :]
                    for sub in range(2):
                        nc.tensor.matmul(
                            ps[:, sub * ST : (sub + 1) * ST],
                            wsl,
                            x8_3d[:, 2 * l2 : 2 * l2 + 2, sub * ST : (sub + 1) * ST],
                            start=(l2 == 0),
                            stop=(l2 == (0 if ABLATE == "no_scores" else LC // 2 - 1)),
                            perf_mode=mybir.MatmulPerfMode.DoubleRow,
                        )
                dst = apool.tile([128, W], bf16, tag=f"act{wg}")
                sc = (1.0 if wg < 2 else 0.5) / W_SCALE
                nc.scalar.activation(
                    dst[:], ps, ACT.Tanh, bias=bcol[:, wg : wg + 1], scale=sc
                )
                acts.append(dst)
            a0, a1, t0, t1 = acts
            m0 = mpool.tile([128, W], bf16, tag="m0")
            nc.vector.scalar_tensor_tensor(m0[:], t0[:], 1.0, a0[:], ALU.add, ALU.mult)
            m1 = mpool.tile([128, W], bf16, tag="m1")
            nc.vector.scalar_tensor_tensor(m1[:], t1[:], 1.0, a1[:], ALU.add, ALU.mult)
            aps_t = ap_ps.tile([128, W], f32, tag="A_p")
            aps = aps_t[:]
            for sub in range(2):
                sl = slice(sub * ST, (sub + 1) * ST)
                nc.tensor.matmul(aps[:, sl], wrep[:, 0:128], m0[:, sl], start=True, stop=False)
                nc.tensor.matmul(aps[:, sl], wrep[:, 128:256], m1[:, sl], start=False, stop=True)
            ubc = upool.tile([128, W], bf16, tag="ubc_p")
            nc.scalar.activation(ubc[:], aps, ACT.Exp, bias=bcs[:, 0:1])
            pool_tail(g, xb, ubc, W, g * W)

        def do_single(q):
            if ABLATE == "no_dma":
                xb = xbc
            else:
                xb = xpbp.tile([128, LC * ST], bf16, tag="xs")
                nc.sync.dma_start(xb[:], xs_d[q].rearrange("p c n -> p (c n)"))
            acts = []
            for wg in range(4):
                ps_full = scp.tile([128, 1024], f32, tag="ps_p")
                ps = ps_full[:, 0:ST]
                for lc in range(1 if ABLATE == "no_scores" else LC):
                    nc.tensor.matmul(
                        ps[:],
                        wb_sb[:, (wg * LC + lc) * 128 : (wg * LC + lc + 1) * 128],
                        xb[:, lc * ST : (lc + 1) * ST],
                        start=(lc == 0),
                        stop=(lc == (0 if ABLATE == "no_scores" else LC - 1)),
                    )
                dst = apool.tile([128, ST], bf16, tag=f"sact{wg}")
                sc = 1.0 if wg < 2 else 0.5
                nc.scalar.activation(
                    dst[:], ps, ACT.Tanh, bias=bcol[:, wg : wg + 1], scale=sc
                )
                acts.append(dst)
            a0, a1, t0, t1 = acts
            m0 = mpool.tile([128, ST], bf16, tag="sm0")
            nc.vector.scalar_tensor_tensor(m0[:], t0[:], 1.0, a0[:], ALU.add, ALU.mult)
            m1 = mpool.tile([128, ST], bf16, tag="sm1")
            nc.vector.scalar_tensor_tensor(m1[:], t1[:], 1.0, a1[:], ALU.add, ALU.mult)
            aps_t = ap_ps.tile([128, 1024], f32, tag="A_p")
            aps = aps_t[:, 0:ST]
            nc.tensor.matmul(aps, wrep[:, 0:128], m0[:], start=True, stop=False)
            nc.tensor.matmul(aps, wrep[:, 128:256], m1[:], start=False, stop=True)
            ubc = upool.tile([128, ST], bf16, tag="ubc_s")
            nc.scalar.activation(ubc[:], aps, ACT.Exp, bias=bcs[:, 0:1])
            pool_tail(P + q, xb, ubc, ST, P * W + q * ST)

        # interleave singles among pairs for smoother engine mix
        order = []
        pi, qi = 0, 0
        for slot in range(P + Q):
            if Q and (slot + 1) * Q // (P + Q) > qi:
                order.append(("s", qi)); qi += 1
            else:
                order.append(("p", pi)); pi += 1
        for _rep in range(reps):
            for kind, idx in order:
                if kind == "p":
                    do_pair(idx)
                else:
                    do_single(idx)

        nc.sync.dma_start(pp_d[:], pp_sb[:])

    _split_multiwait(nc)
    return nc


def _prepare_v3(x, lengths, Wa, ba, Wb, bb, Wc, bc):
    """Pack 512-row slots into per-core structures of P pairs + Q singles.
    Returns (key, in_maps, slot_map); key = ("v3", P, Q)."""
    x = np.asarray(x, dtype=np.float32)
    lengths = np.asarray(lengths, dtype=np.int64)
    Wa = np.asarray(Wa, dtype=np.float32)
    ba = np.asarray(ba, dtype=np.float32)
    Wb = np.asarray(Wb, dtype=np.float32)
    bb = np.asarray(bb, dtype=np.float32)
    Wc = np.asarray(Wc, dtype=np.float32).reshape(D)
    bc = float(np.asarray(bc, dtype=np.float32).reshape(()))

    slots_by_b = []
    for b in range(B):
        ln = int(max(1, min(N, lengths[b])))
        slots_by_b.append(
            [(b, k * ST, min(ST, ln - k * ST)) for k in range((ln + ST - 1) // ST)]
        )
    n_slots = sum(len(s) for s in slots_by_b)
    n_cores = 8
    s_units = max(1, (n_slots + n_cores - 1) // n_cores)
    pairs = []
    singles = []
    for sl in slots_by_b:
        for k in range(0, len(sl) - 1, 2):
            pairs.append((sl[k], sl[k + 1]))
        if len(sl) % 2:
            singles.append(sl[-1])
    P = min(len(pairs) // n_cores, s_units // 2)
    Q = s_units - 2 * P
    used_pairs = pairs[: n_cores * P]
    rem = singles + [s for pr in pairs[n_cores * P :] for s in pr]
    assert len(rem) <= n_cores * Q, (len(rem), P, Q)
    rem += [None] * (n_cores * Q - len(rem))

    # weights (same packing as v2)
    wstack = np.zeros((128, 32, 128), dtype=np.float32)
    for g, Wm in enumerate((Wa, Wb)):
        for dc in range(2):
            for lc in range(LC):
                wstack[:, (g * 2 + dc) * LC + lc, :] = Wm[
                    lc * 128 : (lc + 1) * 128, dc * 128 : (dc + 1) * 128
                ]
    w8 = (wstack * W_SCALE).astype(FP8).reshape(128, 32 * 128)
    wb_ = wstack.astype(BF16).reshape(128, 32 * 128)
    bcol = np.stack(
        [ba[0:128], ba[128:256], bb[0:128] * 0.5, bb[128:256] * 0.5], axis=1
    ).astype(np.float32)
    wrep = np.zeros((128, 2, 128), dtype=np.float32)
    wrep[:, 0, :] = (Wc[0:128] * 0.5)[:, None]
    wrep[:, 1, :] = (Wc[128:256] * 0.5)[:, None]
    wrep = wrep.astype(BF16).reshape(128, 256)
    bcs = np.full((128, 1), bc, dtype=np.float32)

    xb_all = x.astype(BF16)

    def fill(dst, slot, col0, width):
        b, r0, nv = slot
        slab = xb_all[b, r0 : r0 + nv, :]
        dst[:, :, col0 : col0 + nv] = slab.reshape(nv, LC, 128).transpose(2, 1, 0)

    in_maps = []
    slot_map = []
    for c in range(n_cores):
        xpb = np.zeros((P, 128, LC, 1024), dtype=BF16)
        xs = np.zeros((Q, 128, LC, ST), dtype=BF16)
        smap = []
        for g in range(P):
            prA, prB = used_pairs[c * P + g]
            fill(xpb[g], prA, 0, 1024)
            fill(xpb[g], prB, 512, 1024)
            smap.append(("p", prA, prB))
        for q in range(Q):
            slot = rem[c * Q + q]
            if slot is not None:
                fill(xs[q], slot, 0, ST)
            smap.append(("s", slot))
        im = {
            "w8": w8, "wb": wb_, "bcol": bcol, "wrep": wrep, "bcs": bcs,
        }
        if P:
            im["xp8"] = xpb.astype(FP8)
            im["xpb"] = xpb
        if Q:
            im["xs"] = xs
        in_maps.append(im)
        slot_map.append(smap)
    return ("v3", P, Q), in_maps, slot_map


def _assemble_v3(results, key, slot_map):
    _, P, Q = key
    W = 1024
    Pl = np.zeros((B, L), dtype=np.float64)
    Ssum = np.zeros(B, dtype=np.float64)
    for c in range(8):
        pp = np.asarray(results[c]["pp"], dtype=np.float64)
        ur = np.asarray(results[c]["urow"]).astype(np.float64)[0]
        for gidx, entry in enumerate(slot_map[c]):
            if entry[0] == "p":
                _, prA, prB = entry
                b = prA[0]
                uoff = gidx * W
                nv = prA[2] + prB[2]
                # valid u: sub0 prefix prA[2] (=512) then sub1 prefix prB[2]
                Ssum[b] += ur[uoff : uoff + prA[2]].sum()
                Ssum[b] += ur[uoff + 512 : uoff + 512 + prB[2]].sum()
            else:
                slot = entry[1]
                if slot is None:
                    continue
                b, r0, nv = slot
                uoff = P * W + (gidx - P) * ST
                Ssum[b] += ur[uoff : uoff + nv].sum()
            cols = pp[:, gidx * LC : (gidx + 1) * LC]
            Pl[b] += cols.T.reshape(L)
    return (Pl / Ssum[:, None]).astype(np.float32)


# ---------------------------------------------------------------------------
# previous-generation builder (modes "f32r" | "bf16" | "fp8"), kept for A/B
# ---------------------------------------------------------------------------
def _build(S, reps=1, mode=None):
    if mode is None:
        mode = MODE
    if isinstance(S, tuple) and S[0] == "v3":
        return _build_v3(S[1], S[2], reps)
    if mode == "v2":
        return _build_v2(S, reps)
    import concourse.bass as bass
    import concourse.mybir as mybir
    import concourse.tile as tile
    from contextlib import ExitStack

    f32 = mybir.dt.float32
    f32r = mybir.dt.float32r
    bf16 = mybir.dt.bfloat16
    fp8e4 = mybir.dt.float8e4
    ACT = mybir.ActivationFunctionType
    ALU = mybir.AluOpType

    xdt = {"f32r": f32r, "bf16": bf16, "fp8": bf16}[mode]
    wdt = {"f32r": f32r, "bf16": bf16, "fp8": fp8e4}[mode]

    nc = bass.Bass()
    xt_d = nc.declare_dram_parameter("xt", [S, 128, LC, ST], xdt, isOutput=False)
    if mode == "fp8":
        xt8_d = nc.declare_dram_parameter(
            "xt8", [S, 128, LC, ST], fp8e4, isOutput=False
        )
    mrow_d = nc.declare_dram_parameter("mrow", [1, S * ST], f32, isOutput=False)
    wsb_d = nc.declare_dram_parameter("wsb", [128, 32 * 128], wdt, isOutput=False)
    bcol_d = nc.declare_dram_parameter("bcol", [128, 4], f32, isOutput=False)
    wcc_d = nc.declare_dram_parameter("wcc", [128, 2], bf16, isOutput=False)
    bcs_d = nc.declare_dram_parameter("bcs", [1, 1], f32, isOutput=False)
    ones_d = nc.declare_dram_parameter("ones", [1, 128], f32r, isOutput=False)
    pp_d = nc.declare_dram_parameter("pp", [128, S * LC], f32, isOutput=True)
    urow_d = nc.declare_dram_parameter("urow", [1, S * ST], f32r, isOutput=True)

    with tile.TileContext(nc) as tc, ExitStack() as ctx:
        const = ctx.enter_context(tc.tile_pool(name="const", bufs=1))
        outp = ctx.enter_context(tc.tile_pool(name="outp", bufs=1))
        xpool = ctx.enter_context(tc.tile_pool(name="x", bufs=5))
        apool = ctx.enter_context(tc.tile_pool(name="act", bufs=WBUFS))
        mpool = ctx.enter_context(tc.tile_pool(name="m", bufs=WBUFS))
        upool = ctx.enter_context(tc.tile_pool(name="u", bufs=WBUFS))
        spool = ctx.enter_context(tc.tile_pool(name="scr", bufs=WBUFS))
        scp = ctx.enter_context(tc.tile_pool(name="scp", bufs=4, space="PSUM"))
        ap_ps = ctx.enter_context(tc.tile_pool(name="apps", bufs=2, space="PSUM"))
        bc_ps = ctx.enter_context(tc.tile_pool(name="bcps", bufs=2, space="PSUM"))

        w_sb = const.tile([128, 32 * 128], wdt, tag="w")
        nc.sync.dma_start(w_sb[:], wsb_d[:])
        bcol = const.tile([128, 4], f32, tag="bcol")
        nc.sync.dma_start(bcol[:], bcol_d[:])
        wcc = const.tile([128, 2], bf16, tag="wcc")
        nc.sync.dma_start(wcc[:], wcc_d[:])
        bcs = const.tile([1, 1], f32, tag="bcs")
        nc.sync.dma_start(bcs[:], bcs_d[:])
        ones = const.tile([1, 128], f32r, tag="ones")
        nc.sync.dma_start(ones[:], ones_d[:])
        msb = const.tile([1, S * ST], f32, tag="mrow")
        nc.sync.dma_start(msb[:], mrow_d[:])

        pp_sb = outp.tile([128, S * LC], f32, tag="pp")
        u_row = outp.tile([1, S * ST], f32r, tag="urow")

        def scores_part(st):
            xts = xpool.tile([128, LC * ST], xdt, tag="xt")
            nc.sync.dma_start(xts[:], xt_d[st].rearrange("p c n -> p (c n)"))
            if mode == "fp8":
                xts8 = xpool.tile([128, LC * ST], fp8e4, tag="xt8")
                nc.sync.dma_start(xts8[:], xt8_d[st].rearrange("p c n -> p (c n)"))
                xts8_3d = xts8[:].rearrange("p (c n) -> p c n", c=LC)
                w_3d = w_sb[:].rearrange("p (k m) -> p k m", k=32)
            acts = []
            for wg in range(4):
                ps = scp.tile([128, ST], f32, tag="ps")
                if mode == "fp8":
                    for l2 in range(LC // 2):
                        nc.tensor.matmul(
                            ps[:],
                            w_3d[:, wg * LC + 2 * l2 : wg * LC + 2 * l2 + 2, :],
                            xts8_3d[:, 2 * l2 : 2 * l2 + 2, :],
                            start=(l2 == 0),
                            stop=(l2 == (0 if ABLATE == "no_scores" else LC // 2 - 1)),
                            perf_mode=mybir.MatmulPerfMode.DoubleRow,
                        )
                else:
                    for lc in range(LC):
                        nc.tensor.matmul(
                            ps[:],
                            w_sb[:, (wg * LC + lc) * 128 : (wg * LC + lc + 1) * 128],
                            xts[:, lc * ST : (lc + 1) * ST],
                            start=(lc == 0),
                            stop=(lc == LC - 1),
                        )
                dst = apool.tile([128, ST], bf16, tag=f"act{wg}")
                sc = (1.0 if wg < 2 else 0.5) / (W_SCALE if mode == "fp8" else 1.0)
                nc.scalar.activation(
                    dst[:], ps[:], ACT.Tanh, bias=bcol[:, wg : wg + 1], scale=sc
                )
                acts.append(dst)
            return xts, acts

        def tail_part(st, xts, acts):
            a0, a1, t0, t1 = acts
            m0 = mpool.tile([128, ST], bf16, tag="m0")
            nc.vector.scalar_tensor_tensor(m0[:], t0[:], 1.0, a0[:], ALU.add, ALU.mult)
            m1 = mpool.tile([128, ST], bf16, tag="m1")
            nc.vector.scalar_tensor_tensor(m1[:], t1[:], 1.0, a1[:], ALU.add, ALU.mult)
            aps = ap_ps.tile([1, ST], f32, tag="A")
            nc.tensor.matmul(aps[:], wcc[:, 0:1], m0[:], start=True, stop=False)
            nc.tensor.matmul(aps[:], wcc[:, 1:2], m1[:], start=False, stop=True)
            useg = u_row[0:1, st * ST : (st + 1) * ST]
            nc.scalar.activation(useg, aps[:], ACT.Exp, bias=bcs[0:1, 0:1])
            nc.vector.tensor_tensor(
                useg, useg, msb[0:1, st * ST : (st + 1) * ST], ALU.mult
            )
            bps = bc_ps.tile([128, ST], f32, tag="bc")
            nc.tensor.matmul(bps[:], ones[:], useg, start=True, stop=True)
            ubc = upool.tile([128, ST], f32, tag="ubc")
            nc.scalar.copy(ubc[:], bps[:])
            xsrc = xts[:].bitcast(f32) if mode == "f32r" else xts[:]
            for c2 in range(LC):
                to = spool.tile([128, ST], bf16, tag="ttr")
                nc.vector.scalar_tensor_tensor(
                    to[:],
                    xsrc[:, c2 * ST : (c2 + 1) * ST],
                    1.0,
                    ubc[:],
                    ALU.bypass,
                    ALU.mult,
                    accum_out=pp_sb[:, st * LC + c2 : st * LC + c2 + 1],
                )

        for _rep in range(reps):
            for st in range(S):
                tail_part(st, *scores_part(st))

        nc.sync.dma_start(pp_d[:], pp_sb[:])
        nc.sync.dma_start(urow_d[:], u_row[:])

    _split_multiwait(nc)
    return nc


def _get_program(S, reps=1):
    key = (S, reps, MODE, NB, POOL_CHUNKS, PAIRED, LDW_OPT, POOL_NACT, K_GP, ABLATE)
    if key not in _cache:
        if LDW_OPT:
            _patch_ldw_opt()
        _cache[key] = _build(S, reps)
    return _cache[key]


def _prepare_old(x, lengths, Wa, ba, Wb, bb, Wc, bc):
    x = np.asarray(x, dtype=np.float32)
    lengths = np.asarray(lengths, dtype=np.int64)
    Wa = np.asarray(Wa, dtype=np.float32)
    ba = np.asarray(ba, dtype=np.float32)
    Wb = np.asarray(Wb, dtype=np.float32)
    bb = np.asarray(bb, dtype=np.float32)
    Wc = np.asarray(Wc, dtype=np.float32).reshape(D)
    bc = float(np.asarray(bc, dtype=np.float32).reshape(()))

    slots = []
    for b in range(B):
        ln = int(max(1, min(N, lengths[b])))
        for k in range((ln + ST - 1) // ST):
            slots.append((b, k * ST, min(ST, ln - k * ST)))
    n_cores = 8
    S = max(1, (len(slots) + n_cores - 1) // n_cores)

    xdt = {"f32r": np.float32, "bf16": BF16, "fp8": BF16}[MODE]
    wdt = {"f32r": np.float32, "bf16": BF16, "fp8": FP8}[MODE]
    wmul = W_SCALE if MODE == "fp8" else 1.0
    wsb = np.zeros((128, 32, 128), dtype=wdt)
    for g, W in enumerate((Wa, Wb)):
        for dc in range(2):
            for lc in range(LC):
                wsb[:, (g * 2 + dc) * LC + lc, :] = (
                    W[lc * 128 : (lc + 1) * 128, dc * 128 : (dc + 1) * 128] * wmul
                ).astype(wdt)
    wsb = wsb.reshape(128, 32 * 128)
    bcol = np.stack(
        [ba[0:128], ba[128:256], bb[0:128] * 0.5, bb[128:256] * 0.5], axis=1
    ).astype(np.float32)
    wcc = np.stack([Wc[0:128] * 0.5, Wc[128:256] * 0.5], axis=1).astype(BF16)

    in_maps = []
    slot_map = []
    xcast = x if xdt == np.float32 else x.astype(xdt)
    for c in range(n_cores):
        xt = np.zeros((S, 128, LC, ST), dtype=xdt)
        mrow = np.zeros((S, ST), dtype=np.float32)
        smap = []
        for k in range(S):
            g = c * S + k
            if g < len(slots):
                b, r0, nvalid = slots[g]
                slab = xcast[b, r0 : r0 + nvalid, :]
                xt[k, :, :, :nvalid] = slab.reshape(nvalid, LC, 128).transpose(2, 1, 0)
                mrow[k, :nvalid] = 1.0
                smap.append((b, r0, nvalid))
            else:
                smap.append(None)
        im = {
            "xt": xt,
            "mrow": mrow.reshape(1, S * ST),
            "wsb": wsb,
            "bcol": bcol,
            "wcc": wcc,
            "bcs": np.array([[bc]], dtype=np.float32),
            "ones": np.ones((1, 128), dtype=np.float32),
        }
        if MODE == "fp8":
            im["xt8"] = xt.astype(FP8)
        in_maps.append(im)
        slot_map.append(smap)
    return S, in_maps, slot_map


def _prepare(x, lengths, Wa, ba, Wb, bb, Wc, bc):
    if MODE == "v3":
        return _prepare_v3(x, lengths, Wa, ba, Wb, bb, Wc, bc)
    if MODE == "v2":
        return _prepare_v2(x, lengths, Wa, ba, Wb, bb, Wc, bc)
    return _prepare_old(x, lengths, Wa, ba, Wb, bb, Wc, bc)


def _assemble(results, S, slot_map):
    if isinstance(S, tuple) and S[0] == "v3":
        return _assemble_v3(results, S, slot_map)
    return _assemble_v2(results, S, slot_map)


def kernel(x, lengths, Wa, ba, Wb, bb, Wc, bc, _reps=1):
    from concourse.bass_utils import run_bass_kernel_spmd

    S, in_maps, slot_map = _prepare(x, lengths, Wa, ba, Wb, bb, Wc, bc)
    nc = _get_program(S, _reps)
    res = run_bass_kernel_spmd(nc, in_maps, list(range(8)))
    return _assemble(res.results, S, slot_map)

